# revision 26
# baseline (speedup 1.0000x reference)
"""Trainium2 Bass kernel for nn_Attn_17738214933129.

Dense transformer attention block:
  Q/K/V projections from n_loc=2048 -> feat=512 (8 heads x 64),
  structural-bias softmax added to scaled QK^T scores, softmax, PV,
  output projection back to n_loc=2048.

Sharding: data-parallel over batch (16 -> 2 per core) across 8 NeuronCores,
weights replicated, no collectives.

Layout strategy (per core, rows = 2*512 = 1024):
  - q/k/v are uploaded in bf16 and loaded transposed ([nloc, rows]) via the
    16-bit DMA-transpose xbar (all on the SP HWDGE ring, which carries ONLY
    transposes to avoid xbar-mode transitions), so the contraction dim lands
    on partitions with no input-side PE transposes.
  - Weights and str/mask are uploaded pre-tiled so each is a single large
    plain DMA on the ACT HWDGE ring (runs parallel to the transpose ring).
  - QT[f, r] = sum_nl WqT[nl, f] * qT[nl, r]   (Wq pre-scaled by 1/DH on
    host); KT and VT likewise; V[r, f] is recovered from VT with packed PE
    transposes (4 per PSUM bank, one copy out per bank).
  - Scores S[q, k] = QT_h^T @ KT_h per (b, h); the structural softmax sm is
    added INTO PSUM with an identity-matmul accumulate; E = exp(S + sm) on
    ACT with fused row-sum accumulation; P = E * (1/rowsum) on DVE;
    P transposed on the PE (packed); yT accumulates head pairs straight into
    the output-projection lhsT layout; out = xT^T @ WoT (+ bo as a rank-1
    ones x bias matmul only when biases are nonzero).
"""

import sys

import numpy as np

try:
    import concourse.bass as bass  # noqa: F401
except Exception:  # pragma: no cover - path fallback
    sys.path.insert(0, "/opt/trn_rl_repo")

import ml_dtypes

import concourse.bacc as bacc
import concourse.tile as tile
from concourse import mybir
from concourse.bass_utils import run_bass_kernel_spmd

BF16 = mybir.dt.bfloat16
F32 = mybir.dt.float32
AF = mybir.ActivationFunctionType
ALU = mybir.AluOpType

B, S, NLOC = 16, 512, 2048
FEAT, H, DH = 512, 8, 64
NCORES = 8
BL = B // NCORES          # batch per core = 2
R = BL * S                # rows per core = 1024
KT_N = NLOC // 128        # 16 contraction tiles for projections
FT_N = FEAT // 128        # 4 feature tiles
QT_N = S // 128           # 4 query tiles per batch element
NL_N = NLOC // 512        # 4 output column chunks

_CACHE = {}


def _build(use_bias):
    nc = bacc.Bacc(
        "TRN2",
        target_bir_lowering=False,
        debug=False,
        enable_asserts=False,
        num_devices=NCORES,
    )

    # q/k/v pre-transposed and pre-tiled on host: [128, i*R + r] = x[r, i*128+p].
    d_q = nc.dram_tensor("q", [128, KT_N * R], BF16, kind="ExternalInput").ap()
    d_k = nc.dram_tensor("k", [128, KT_N * R], BF16, kind="ExternalInput").ap()
    d_v = nc.dram_tensor("v", [128, KT_N * R], BF16, kind="ExternalInput").ap()
    # str/mask pre-tiled: [128, BL*QT_N*512] with [p, (b*4+qt)*512+c].
    d_str = nc.dram_tensor("strm", [128, BL * QT_N * S], F32, kind="ExternalInput").ap()
    d_mask = nc.dram_tensor("maskf", [128, BL * QT_N * S], F32, kind="ExternalInput").ap()
    # weights pre-tiled: wq/wk/wv [128, 16*512] with [p, i*512+f]=W.T[i*128+p, f];
    # wo [128, 4*2048] with [p, ft*2048+n]=Wo.T[ft*128+p, n].
    d_wq = nc.dram_tensor("wqT", [128, KT_N * FEAT], BF16, kind="ExternalInput").ap()
    d_wk = nc.dram_tensor("wkT", [128, KT_N * FEAT], BF16, kind="ExternalInput").ap()
    d_wv = nc.dram_tensor("wvT", [128, KT_N * FEAT], BF16, kind="ExternalInput").ap()
    d_wo = nc.dram_tensor("woT", [128, FT_N * NLOC], BF16, kind="ExternalInput").ap()
    d_bq = nc.dram_tensor("bqr", [1, FEAT], BF16, kind="ExternalInput").ap()
    d_bk = nc.dram_tensor("bkr", [1, FEAT], BF16, kind="ExternalInput").ap()
    d_bv = nc.dram_tensor("bvr", [1, FEAT], BF16, kind="ExternalInput").ap()
    d_bo = nc.dram_tensor("bor", [1, NLOC], BF16, kind="ExternalInput").ap()
    d_id = nc.dram_tensor("ident", [128, 128], BF16, kind="ExternalInput").ap()
    d_ones = nc.dram_tensor("onesr", [1, 512], BF16, kind="ExternalInput").ap()
    d_out = nc.dram_tensor("out", [R, NLOC], F32, kind="ExternalOutput").ap()

    with tile.TileContext(nc) as tc:
        with (
            tc.tile_pool(name="consts", bufs=1) as cpool,
            tc.tile_pool(name="weights", bufs=1) as wpool,
            tc.tile_pool(name="persist", bufs=1) as ppool,
            tc.tile_pool(name="qkvT", bufs=8) as spool,
            tc.tile_pool(name="smwork", bufs=1) as mpool,
            tc.tile_pool(name="smcol", bufs=4) as colpool,
            tc.tile_pool(name="attn", bufs=6) as apool,
            tc.tile_pool(name="xout", bufs=2) as xpool,
            tc.tile_pool(name="ptp", bufs=2) as ptpool,
            tc.tile_pool(name="ostage", bufs=2) as opool,
            tc.tile_pool(name="psum", bufs=8, space="PSUM") as psum,
        ):
            # ---- constants ----
            ident = cpool.tile([128, 128], BF16, tag="ident", name="ident")
            ones = cpool.tile([1, 512], BF16, tag="ones", name="ones")
            biases = {}
            if use_bias:
                nc.sync.dma_start(ident[:], d_id[:])
                nc.sync.dma_start(ones[:], d_ones[:])
                for nm, dr, width in (
                    ("bq", d_bq, FEAT),
                    ("bk", d_bk, FEAT),
                    ("bv", d_bv, FEAT),
                    ("bo", d_bo, NLOC),
                ):
                    t = cpool.tile([1, width], BF16, tag=nm, name=nm)
                    nc.sync.dma_start(t[:], dr[:])
                    biases[nm] = t


            # Persistent activations.
            QT = [ppool.tile([128, R], BF16, tag=f"QT{i}", name=f"QT{i}") for i in range(FT_N)]
            KTt = [ppool.tile([128, R], BF16, tag=f"KT{i}", name=f"KT{i}") for i in range(FT_N)]
            V = [ppool.tile([128, FEAT], BF16, tag=f"V{i}", name=f"V{i}") for i in range(R // 128)]

            def projection_T(dst, d_src, w, bias_nm, wdma=None):
                """dst[f, r] tiles: transposed projection (Q, K and VT).

                wdma: optional (d_w, n_chunks) to stream the weight load in
                chunks interleaved with the input stream (keeps the ring FIFO
                from front-loading the whole weight before the first tile).
                """
                groups = {}
                for ft in range(FT_N):
                    for rc in range(R // 512):
                        ps = psum.tile([128, 512], F32, tag="ps", name="ps")
                        if use_bias:
                            nc.tensor.matmul(
                                ps[:],
                                lhsT=biases[bias_nm][0:1, ft * 128 : (ft + 1) * 128],
                                rhs=ones[0:1, :],
                                start=True,
                                stop=False,
                            )
                        groups[(ft, rc)] = ps
                for i in range(KT_N):
                    if wdma is not None:
                        d_w, nch = wdma
                        per = KT_N // nch
                        if i % per == 0:
                            c0 = (i // per) * per * FEAT
                            c1 = c0 + per * FEAT
                            nc.sync.dma_start(w[:, c0:c1], d_w[:, c0:c1])
                    xt = spool.tile([128, R], BF16, tag="xT", name="xt_in")
                    nc.sync.dma_start(xt[:], d_src[:, i * R : (i + 1) * R])
                    for ft in range(FT_N):
                        for rc in range(R // 512):
                            nc.tensor.matmul(
                                groups[(ft, rc)][:],
                                lhsT=w[:, i * FEAT + ft * 128 : i * FEAT + (ft + 1) * 128],
                                rhs=xt[:, rc * 512 : (rc + 1) * 512],
                                start=(i == 0 and not use_bias),
                                stop=(i == KT_N - 1),
                            )
                for ft in range(FT_N):
                    for rc in range(R // 512):
                        nc.vector.tensor_copy(
                            dst[ft][:, rc * 512 : (rc + 1) * 512],
                            groups[(ft, rc)][:],
                        )

            wq = wpool.tile([128, KT_N * FEAT], BF16, tag="wq", name="wq")
            projection_T(QT, d_q, wq, "bq", wdma=(d_wq, 8))
            wk = wpool.tile([128, KT_N * FEAT], BF16, tag="wk", name="wk")
            if not use_bias:
                nc.sync.dma_start(ident[:], d_id[:])
                nc.sync.dma_start(ones[:], d_ones[:])
            projection_T(KTt, d_k, wk, "bk", wdma=(d_wk, 4))

            # V projection: VT[f, r] like Q/K (vT stream is the last transpose use).
            wv = wpool.tile([128, KT_N * FEAT], BF16, tag="wv", name="wv")
            VT = [ppool.tile([128, R], BF16, tag=f"VT{i}", name=f"VT{i}") for i in range(FT_N)]
            projection_T(VT, d_v, wv, "bv", wdma=(d_wv, 4))

            wo = wpool.tile([128, FT_N * NLOC], BF16, tag="wo", name="wo")
            nc.sync.dma_start(wo[:], d_wo[:])

            # ---- structural softmax for both batch elements (bf16 out) ---
            sm_all = {}
            for b in range(BL):
                stb = mpool.tile([128, QT_N * S], F32, tag="strb", name="strb")
                nc.sync.dma_start(
                    stb[:], d_str[:, b * QT_N * S : (b + 1) * QT_N * S]
                )
                mkb = mpool.tile([128, QT_N * S], F32, tag="maskb", name="maskb")
                nc.sync.dma_start(
                    mkb[:], d_mask[:, b * QT_N * S : (b + 1) * QT_N * S]
                )
                usum = colpool.tile([128, QT_N], F32, tag="usum", name="usum")
                utiles = []
                for qt in range(QT_N):
                    ex = mpool.tile([128, S], F32, tag="expstr", name="expstr")
                    nc.scalar.activation(
                        ex[:], stb[:, qt * S : (qt + 1) * S], AF.Exp
                    )
                    u = mpool.tile([128, S], F32, tag=f"u{qt}", name=f"u{qt}")
                    nc.vector.tensor_tensor(
                        u[:], ex[:], mkb[:, qt * S : (qt + 1) * S], op=ALU.mult
                    )
                    nc.vector.reduce_sum(
                        usum[:, qt : qt + 1], u[:], axis=mybir.AxisListType.X
                    )
                    utiles.append(u)
                ru = colpool.tile([128, QT_N], F32, tag="ru", name="ru")
                nc.vector.reciprocal(ru[:], usum[:])
                for qt in range(QT_N):
                    t = mpool.tile([128, S], BF16, tag=f"smb{qt}", name=f"smb{qt}")
                    nc.vector.tensor_scalar(
                        t[:], utiles[qt][:], ru[:, qt : qt + 1], None, op0=ALU.mult
                    )
                    sm_all[(b, qt)] = t

            # Packed PE transposes: VT -> V.
            for rt in range(R // 128):
                tp = psum.tile([128, 512], F32, tag="ps", name="vtp")
                for ft in range(FT_N):
                    nc.tensor.matmul(
                        tp[:, ft * 128 : (ft + 1) * 128],
                        lhsT=VT[ft][:, rt * 128 : (rt + 1) * 128],
                        rhs=ident[:],
                        start=(ft == 0),
                        stop=(ft == FT_N - 1),
                    )
                nc.vector.tensor_copy(V[rt][:], tp[:])

            # ---- attention + output projection ---------------------------
            for b in range(BL):
                xT = [xpool.tile([128, S], BF16, tag=f"xT{j}", name=f"xTo{j}") for j in range(FT_N)]
                yps = None
                for hp in range(H // 2):
                    ht = hp
                    ssum = colpool.tile([128, 2 * QT_N], F32, tag="ssum", name="ssum")
                    rs = colpool.tile([128, 2 * QT_N], F32, tag="rs", name="rs")
                    # Scores for the head pair, interleaved so the PE row-tiles
                    # (even head rows 0-63, odd head rows 64-127) run together.
                    spairs = {}
                    for qt in range(QT_N):
                        for hs in range(2):
                            hb = hs * 64
                            sps = psum.tile([128, 512], F32, tag="ps", name="ps")
                            nc.tensor.matmul(
                                sps[:],
                                lhsT=QT[ht][
                                    hb : hb + 64,
                                    b * S + qt * 128 : b * S + (qt + 1) * 128,
                                ],
                                rhs=KTt[ht][hb : hb + 64, b * S : (b + 1) * S],
                                start=True,
                                stop=False,
                            )
                            spairs[(hs, qt)] = sps
                        for hs in range(2):
                            nc.tensor.matmul(
                                spairs[(hs, qt)][:],
                                lhsT=ident[:],
                                rhs=sm_all[(b, qt)][:],
                                start=False,
                                stop=True,
                            )
                    ppair = {}
                    for qt in range(QT_N):
                        for hs in range(2):
                            col = hs * QT_N + qt
                            e = apool.tile([128, S], BF16, tag="etile", name="etile")
                            nc.scalar.activation(
                                e[:],
                                spairs[(hs, qt)][:],
                                AF.Exp,
                                accum_out=ssum[:, col : col + 1],
                            )
                            nc.vector.reciprocal(
                                rs[:, col : col + 1], ssum[:, col : col + 1]
                            )
                            p = apool.tile([128, S], BF16, tag="ptile", name="ptile")
                            nc.vector.tensor_scalar(
                                p[:], e[:], rs[:, col : col + 1], None, op0=ALU.mult
                            )
                            ppair[(hs, qt)] = p
                    PTp = {}
                    for hs in range(2):
                        PTp[hs] = [
                            ptpool.tile([128, S], BF16, tag=f"PT{hs}{kt}", name=f"PT{hs}{kt}")
                            for kt in range(QT_N)
                        ]
                        for kt in range(QT_N):
                            tp = psum.tile([128, 512], F32, tag="ps", name="pst")
                            for qt in range(QT_N):
                                nc.tensor.matmul(
                                    tp[:, qt * 128 : (qt + 1) * 128],
                                    lhsT=ppair[(hs, qt)][:, kt * 128 : (kt + 1) * 128],
                                    rhs=ident[:],
                                    start=(qt == 0),
                                    stop=(qt == QT_N - 1),
                                )
                            nc.vector.tensor_copy(PTp[hs][kt][:], tp[:])
                    # PV for the pair, interleaved so the PE col-tiles
                    # (even head cols 0-63, odd head cols 64-127) run together.
                    yps = psum.tile([128, 512], F32, tag="ps", name="ps")
                    for kt in range(QT_N):
                        for hs in range(2):
                            h = 2 * hp + hs
                            hb = hs * 64
                            nc.tensor.matmul(
                                yps[hb : hb + 64, :],
                                lhsT=V[b * QT_N + kt][:, h * 64 : (h + 1) * 64],
                                rhs=PTp[hs][kt][:],
                                start=(kt == 0),
                                stop=(kt == QT_N - 1),
                            )
                    nc.vector.tensor_copy(xT[ht][:], yps[:])

                # Output projection for this batch element.
                for qt in range(QT_N):
                    row0 = b * S + qt * 128
                    ot = opool.tile([128, NLOC], F32, tag="ot", name="ot")
                    for nlc in range(NL_N):
                        ps = psum.tile([128, 512], F32, tag="ps", name="ps")
                        if use_bias:
                            nc.tensor.matmul(
                                ps[:],
                                lhsT=ones[0:1, 0:128],
                                rhs=biases["bo"][0:1, nlc * 512 : (nlc + 1) * 512],
                                start=True,
                                stop=False,
                            )
                        for ft in range(FT_N):
                            nc.tensor.matmul(
                                ps[:],
                                lhsT=xT[ft][:, qt * 128 : (qt + 1) * 128],
                                rhs=wo[:, ft * NLOC + nlc * 512 : ft * NLOC + (nlc + 1) * 512],
                                start=(ft == 0 and not use_bias),
                                stop=(ft == FT_N - 1),
                            )
                        nc.vector.tensor_copy(
                            ot[:, nlc * 512 : (nlc + 1) * 512], ps[:]
                        )
                        if nlc % 2 == 1:
                            nc.sync.dma_start(
                                d_out[row0 : row0 + 128, (nlc - 1) * 512 : (nlc + 1) * 512],
                                ot[:, (nlc - 1) * 512 : (nlc + 1) * 512],
                            )

    nc.compile()
    return nc


def _prep_inputs(q, k, v, str_mat, attn_mask, Wq, bq, Wk, bk, Wv, bv, Wo, bo):
    bf = ml_dtypes.bfloat16
    # 1/DH folded into Wq (and bq): DH = 64 = 2^6, exact in floating point.
    wqT = np.ascontiguousarray((Wq / np.float32(DH)).T).astype(bf)
    wkT = np.ascontiguousarray(Wk.T).astype(bf)
    wvT = np.ascontiguousarray(Wv.T).astype(bf)
    woT = np.ascontiguousarray(Wo.T).astype(bf)

    # Pre-tile weights: [n*128, width] -> [128, n*width].
    def pretile(w):
        n = w.shape[0] // 128
        return np.ascontiguousarray(
            w.reshape(n, 128, w.shape[1]).transpose(1, 0, 2).reshape(128, -1)
        )

    wqt = pretile(wqT)
    wkt = pretile(wkT)
    wvt = pretile(wvT)
    wot = pretile(woT)

    bqr = (bq[None, :] / np.float32(DH)).astype(bf)
    bkr = bk[None, :].astype(bf)
    bvr = bv[None, :].astype(bf)
    bor = bo[None, :].astype(bf)
    ident = np.eye(128, dtype=bf)
    onesr = np.ones((1, 512), dtype=bf)

    q16 = np.asarray(q).astype(bf)
    k16 = np.asarray(k).astype(bf)
    v16 = np.asarray(v).astype(bf)

    def pretile_T(x):
        # [R, NLOC] -> [128, KT_N*R] with [p, i*R+r] = x[r, i*128+p]
        return np.ascontiguousarray(
            x.reshape(R, KT_N, 128).transpose(2, 1, 0).reshape(128, KT_N * R)
        )
    strf = np.asarray(str_mat, dtype=np.float32)
    maskf = np.asarray(attn_mask).astype(np.float32)

    in_maps = []
    for c in range(NCORES):
        sl = slice(c * BL, (c + 1) * BL)
        # [BL, S, S] -> [128, BL*QT_N*S] with [p, (b*4+qt)*S+col].
        strt = np.ascontiguousarray(
            strf[sl].reshape(BL * QT_N, 128, S).transpose(1, 0, 2).reshape(128, -1)
        )
        maskt = np.ascontiguousarray(
            maskf[sl].reshape(BL * QT_N, 128, S).transpose(1, 0, 2).reshape(128, -1)
        )
        in_maps.append(
            {
                "q": pretile_T(q16[sl].reshape(R, NLOC)),
                "k": pretile_T(k16[sl].reshape(R, NLOC)),
                "v": pretile_T(v16[sl].reshape(R, NLOC)),
                "strm": strt,
                "maskf": maskt,
                "wqT": wqt,
                "wkT": wkt,
                "wvT": wvt,
                "woT": wot,
                "bqr": bqr,
                "bkr": bkr,
                "bvr": bvr,
                "bor": bor,
                "ident": ident,
                "onesr": onesr,
            }
        )
    return in_maps


def kernel(q, k, v, str_mat, attn_mask, Wq, bq, Wk, bk, Wv, bv, Wo, bo):
    use_bias = bool(
        np.any(np.asarray(bq))
        or np.any(np.asarray(bk))
        or np.any(np.asarray(bv))
        or np.any(np.asarray(bo))
    )
    key = ("nc", use_bias)
    if key not in _CACHE:
        _CACHE[key] = _build(use_bias)
    nc = _CACHE[key]
    in_maps = _prep_inputs(
        q, k, v, str_mat, attn_mask, Wq, bq, Wk, bk, Wv, bv, Wo, bo
    )
    res = run_bass_kernel_spmd(nc, in_maps, core_ids=list(range(NCORES)))
    out = np.empty((B, S, NLOC), dtype=np.float32)
    for c in range(NCORES):
        out[c * BL : (c + 1) * BL] = res.results[c]["out"].reshape(BL, S, NLOC)
    return out


# revision 27
# speedup vs baseline: 1.0650x; 1.0650x over previous
"""Trainium2 Bass kernel for nn_Attn_17738214933129.

Dense transformer attention block:
  Q/K/V projections from n_loc=2048 -> feat=512 (8 heads x 64),
  structural-bias softmax added to scaled QK^T scores, softmax, PV,
  output projection back to n_loc=2048.

Sharding: data-parallel over batch (16 -> 2 per core) across 8 NeuronCores,
weights replicated, no collectives.

Layout strategy (per core, rows = 2*512 = 1024):
  - q/k/v are uploaded in bf16 and loaded transposed ([nloc, rows]) via the
    16-bit DMA-transpose xbar (all on the SP HWDGE ring, which carries ONLY
    transposes to avoid xbar-mode transitions), so the contraction dim lands
    on partitions with no input-side PE transposes.
  - Weights and str/mask are uploaded pre-tiled so each is a single large
    plain DMA on the ACT HWDGE ring (runs parallel to the transpose ring).
  - QT[f, r] = sum_nl WqT[nl, f] * qT[nl, r]   (Wq pre-scaled by 1/DH on
    host); KT and VT likewise; V[r, f] is recovered from VT with packed PE
    transposes (4 per PSUM bank, one copy out per bank).
  - Scores S[q, k] = QT_h^T @ KT_h per (b, h); the structural softmax sm is
    added INTO PSUM with an identity-matmul accumulate; E = exp(S + sm) on
    ACT with fused row-sum accumulation; P = E * (1/rowsum) on DVE;
    P transposed on the PE (packed); yT accumulates head pairs straight into
    the output-projection lhsT layout; out = xT^T @ WoT (+ bo as a rank-1
    ones x bias matmul only when biases are nonzero).
"""

import sys

import numpy as np

try:
    import concourse.bass as bass  # noqa: F401
except Exception:  # pragma: no cover - path fallback
    sys.path.insert(0, "/opt/trn_rl_repo")

import ml_dtypes

import concourse.bacc as bacc
import concourse.tile as tile
from concourse import mybir
from concourse.bass_utils import run_bass_kernel_spmd

BF16 = mybir.dt.bfloat16
F32 = mybir.dt.float32
AF = mybir.ActivationFunctionType
ALU = mybir.AluOpType

B, S, NLOC = 16, 512, 2048
FEAT, H, DH = 512, 8, 64
NCORES = 8
BL = B // NCORES          # batch per core = 2
R = BL * S                # rows per core = 1024
KT_N = NLOC // 128        # 16 contraction tiles for projections
FT_N = FEAT // 128        # 4 feature tiles
QT_N = S // 128           # 4 query tiles per batch element
NL_N = NLOC // 512        # 4 output column chunks

_CACHE = {}


def _build(use_bias):
    nc = bacc.Bacc(
        "TRN2",
        target_bir_lowering=False,
        debug=False,
        enable_asserts=False,
        num_devices=NCORES,
    )

    # q/k/v pre-transposed and pre-tiled on host: [128, i*R + r] = x[r, i*128+p].
    d_q = nc.dram_tensor("q", [128, KT_N * R], BF16, kind="ExternalInput").ap()
    d_k = nc.dram_tensor("k", [128, KT_N * R], BF16, kind="ExternalInput").ap()
    d_v = nc.dram_tensor("v", [128, KT_N * R], BF16, kind="ExternalInput").ap()
    # str/mask pre-tiled: [128, BL*QT_N*512] with [p, (b*4+qt)*512+c].
    d_str = nc.dram_tensor("strm", [128, BL * QT_N * S], F32, kind="ExternalInput").ap()
    d_mask = nc.dram_tensor("maskf", [128, BL * QT_N * S], F32, kind="ExternalInput").ap()
    # weights pre-tiled: wq/wk/wv [128, 16*512] with [p, i*512+f]=W.T[i*128+p, f];
    # wo [128, 4*2048] with [p, ft*2048+n]=Wo.T[ft*128+p, n].
    d_wq = nc.dram_tensor("wqT", [128, KT_N * FEAT], BF16, kind="ExternalInput").ap()
    d_wk = nc.dram_tensor("wkT", [128, KT_N * FEAT], BF16, kind="ExternalInput").ap()
    d_wv = nc.dram_tensor("wvT", [128, KT_N * FEAT], BF16, kind="ExternalInput").ap()
    d_wo = nc.dram_tensor("woT", [128, FT_N * NLOC], BF16, kind="ExternalInput").ap()
    d_bq = nc.dram_tensor("bqr", [1, FEAT], BF16, kind="ExternalInput").ap()
    d_bk = nc.dram_tensor("bkr", [1, FEAT], BF16, kind="ExternalInput").ap()
    d_bv = nc.dram_tensor("bvr", [1, FEAT], BF16, kind="ExternalInput").ap()
    d_bo = nc.dram_tensor("bor", [1, NLOC], BF16, kind="ExternalInput").ap()
    d_id = nc.dram_tensor("ident", [128, 128], BF16, kind="ExternalInput").ap()
    d_ones = nc.dram_tensor("onesr", [1, 512], BF16, kind="ExternalInput").ap()
    d_out = nc.dram_tensor("out", [R, NLOC], F32, kind="ExternalOutput").ap()

    with tile.TileContext(nc) as tc:
        with (
            tc.tile_pool(name="consts", bufs=1) as cpool,
            tc.tile_pool(name="weights", bufs=1) as wpool,
            tc.tile_pool(name="persist", bufs=1) as ppool,
            tc.tile_pool(name="qkvT", bufs=8) as spool,
            tc.tile_pool(name="smwork", bufs=1) as mpool,
            tc.tile_pool(name="smcol", bufs=4) as colpool,
            tc.tile_pool(name="attn", bufs=6) as apool,
            tc.tile_pool(name="xout", bufs=2) as xpool,
            tc.tile_pool(name="ptp", bufs=2) as ptpool,
            tc.tile_pool(name="ostage", bufs=2) as opool,
            tc.tile_pool(name="psum", bufs=8, space="PSUM") as psum,
        ):
            # ---- constants ----
            ident = cpool.tile([128, 128], BF16, tag="ident", name="ident")
            ones = cpool.tile([1, 512], BF16, tag="ones", name="ones")
            biases = {}
            if use_bias:
                nc.sync.dma_start(ident[:], d_id[:])
                nc.sync.dma_start(ones[:], d_ones[:])
                for nm, dr, width in (
                    ("bq", d_bq, FEAT),
                    ("bk", d_bk, FEAT),
                    ("bv", d_bv, FEAT),
                    ("bo", d_bo, NLOC),
                ):
                    t = cpool.tile([1, width], BF16, tag=nm, name=nm)
                    nc.sync.dma_start(t[:], dr[:])
                    biases[nm] = t


            # Persistent activations.
            QT = [ppool.tile([128, R], BF16, tag=f"QT{i}", name=f"QT{i}") for i in range(FT_N)]
            KTt = [ppool.tile([128, R], BF16, tag=f"KT{i}", name=f"KT{i}") for i in range(FT_N)]
            V = [ppool.tile([128, FEAT], BF16, tag=f"V{i}", name=f"V{i}") for i in range(R // 128)]

            def projection_T(dst, d_src, w, bias_nm, wdma=None):
                """dst[f, r] tiles: transposed projection (Q, K and VT).

                wdma: optional (d_w, n_chunks) to stream the weight load in
                chunks interleaved with the input stream (keeps the ring FIFO
                from front-loading the whole weight before the first tile).
                """
                groups = {}
                for ft in range(FT_N):
                    for rc in range(R // 512):
                        ps = psum.tile([128, 512], F32, tag="ps", name="ps")
                        if use_bias:
                            nc.tensor.matmul(
                                ps[:],
                                lhsT=biases[bias_nm][0:1, ft * 128 : (ft + 1) * 128],
                                rhs=ones[0:1, :],
                                start=True,
                                stop=False,
                            )
                        groups[(ft, rc)] = ps
                for i in range(KT_N):
                    if wdma is not None:
                        d_w, nch = wdma
                        per = KT_N // nch
                        if i % per == 0:
                            c0 = (i // per) * per * FEAT
                            c1 = c0 + per * FEAT
                            nc.sync.dma_start(w[:, c0:c1], d_w[:, c0:c1])
                    xt = spool.tile([128, R], BF16, tag="xT", name="xt_in")
                    nc.sync.dma_start(xt[:], d_src[:, i * R : (i + 1) * R])
                    for ft in range(FT_N):
                        for rc in range(R // 512):
                            nc.tensor.matmul(
                                groups[(ft, rc)][:],
                                lhsT=w[:, i * FEAT + ft * 128 : i * FEAT + (ft + 1) * 128],
                                rhs=xt[:, rc * 512 : (rc + 1) * 512],
                                start=(i == 0 and not use_bias),
                                stop=(i == KT_N - 1),
                            )
                for ft in range(FT_N):
                    for rc in range(R // 512):
                        nc.vector.tensor_copy(
                            dst[ft][:, rc * 512 : (rc + 1) * 512],
                            groups[(ft, rc)][:],
                        )

            wq = wpool.tile([128, KT_N * FEAT], BF16, tag="wq", name="wq")
            projection_T(QT, d_q, wq, "bq", wdma=(d_wq, 8))
            wk = wpool.tile([128, KT_N * FEAT], BF16, tag="wk", name="wk")
            if not use_bias:
                nc.sync.dma_start(ident[:], d_id[:])
                nc.sync.dma_start(ones[:], d_ones[:])
            projection_T(KTt, d_k, wk, "bk", wdma=(d_wk, 4))

            # V projection: VT[f, r] like Q/K (vT stream is the last transpose use).
            wv = wpool.tile([128, KT_N * FEAT], BF16, tag="wv", name="wv")
            VT = [ppool.tile([128, R], BF16, tag=f"VT{i}", name=f"VT{i}") for i in range(FT_N)]
            projection_T(VT, d_v, wv, "bv", wdma=(d_wv, 4))

            wo = wpool.tile([128, FT_N * NLOC], BF16, tag="wo", name="wo")
            nc.sync.dma_start(wo[:], d_wo[:])

            # ---- structural softmax for both batch elements (bf16 out) ---
            sm_all = {}
            for b in range(BL):
                stb = mpool.tile([128, QT_N * S], F32, tag="strb", name="strb")
                nc.sync.dma_start(
                    stb[:], d_str[:, b * QT_N * S : (b + 1) * QT_N * S]
                )
                mkb = mpool.tile([128, QT_N * S], F32, tag="maskb", name="maskb")
                nc.sync.dma_start(
                    mkb[:], d_mask[:, b * QT_N * S : (b + 1) * QT_N * S]
                )
                usum = colpool.tile([128, QT_N], F32, tag="usum", name="usum")
                utiles = []
                for qt in range(QT_N):
                    ex = mpool.tile([128, S], F32, tag="expstr", name="expstr")
                    nc.scalar.activation(
                        ex[:], stb[:, qt * S : (qt + 1) * S], AF.Exp
                    )
                    u = mpool.tile([128, S], F32, tag=f"u{qt}", name=f"u{qt}")
                    nc.vector.tensor_tensor(
                        u[:], ex[:], mkb[:, qt * S : (qt + 1) * S], op=ALU.mult
                    )
                    nc.vector.reduce_sum(
                        usum[:, qt : qt + 1], u[:], axis=mybir.AxisListType.X
                    )
                    utiles.append(u)
                ru = colpool.tile([128, QT_N], F32, tag="ru", name="ru")
                nc.vector.reciprocal(ru[:], usum[:])
                for qt in range(QT_N):
                    t = mpool.tile([128, S], BF16, tag=f"smb{qt}", name=f"smb{qt}")
                    nc.vector.tensor_scalar(
                        t[:], utiles[qt][:], ru[:, qt : qt + 1], None, op0=ALU.mult
                    )
                    sm_all[(b, qt)] = t

            # Packed PE transposes: VT -> V.
            for rt in range(R // 128):
                tp = psum.tile([128, 512], BF16, tag="ps", name="vtp")
                for ft in range(FT_N):
                    nc.tensor.matmul(
                        tp[:, ft * 128 : (ft + 1) * 128],
                        lhsT=VT[ft][:, rt * 128 : (rt + 1) * 128],
                        rhs=ident[:],
                        is_transpose=True,
                        start=(ft == 0),
                        stop=(ft == FT_N - 1),
                    )
                nc.vector.tensor_copy(V[rt][:], tp[:])

            # ---- attention + output projection ---------------------------
            for b in range(BL):
                xT = [xpool.tile([128, S], BF16, tag=f"xT{j}", name=f"xTo{j}") for j in range(FT_N)]
                yps = None
                for hp in range(H // 2):
                    ht = hp
                    ssum = colpool.tile([128, 2 * QT_N], F32, tag="ssum", name="ssum")
                    rs = colpool.tile([128, 2 * QT_N], F32, tag="rs", name="rs")
                    # Scores for the head pair, interleaved so the PE row-tiles
                    # (even head rows 0-63, odd head rows 64-127) run together.
                    spairs = {}
                    for qt in range(QT_N):
                        for hs in range(2):
                            hb = hs * 64
                            sps = psum.tile([128, 512], F32, tag="ps", name="ps")
                            nc.tensor.matmul(
                                sps[:],
                                lhsT=QT[ht][
                                    hb : hb + 64,
                                    b * S + qt * 128 : b * S + (qt + 1) * 128,
                                ],
                                rhs=KTt[ht][hb : hb + 64, b * S : (b + 1) * S],
                                start=True,
                                stop=False,
                            )
                            spairs[(hs, qt)] = sps
                        for hs in range(2):
                            nc.tensor.matmul(
                                spairs[(hs, qt)][:],
                                lhsT=ident[:],
                                rhs=sm_all[(b, qt)][:],
                                start=False,
                                stop=True,
                            )
                    ppair = {}
                    for qt in range(QT_N):
                        for hs in range(2):
                            col = hs * QT_N + qt
                            e = apool.tile([128, S], BF16, tag="etile", name="etile")
                            nc.scalar.activation(
                                e[:],
                                spairs[(hs, qt)][:],
                                AF.Exp,
                                accum_out=ssum[:, col : col + 1],
                            )
                            nc.vector.reciprocal(
                                rs[:, col : col + 1], ssum[:, col : col + 1]
                            )
                            p = apool.tile([128, S], BF16, tag="ptile", name="ptile")
                            nc.vector.tensor_scalar(
                                p[:], e[:], rs[:, col : col + 1], None, op0=ALU.mult
                            )
                            ppair[(hs, qt)] = p
                    PTp = {}
                    for hs in range(2):
                        PTp[hs] = [
                            ptpool.tile([128, S], BF16, tag=f"PT{hs}{kt}", name=f"PT{hs}{kt}")
                            for kt in range(QT_N)
                        ]
                        for kt in range(QT_N):
                            tp = psum.tile([128, 512], BF16, tag="ps", name="pst")
                            for qt in range(QT_N):
                                nc.tensor.matmul(
                                    tp[:, qt * 128 : (qt + 1) * 128],
                                    lhsT=ppair[(hs, qt)][:, kt * 128 : (kt + 1) * 128],
                                    rhs=ident[:],
                                    is_transpose=True,
                                    start=(qt == 0),
                                    stop=(qt == QT_N - 1),
                                )
                            nc.vector.tensor_copy(PTp[hs][kt][:], tp[:])
                    # PV for the pair, interleaved so the PE col-tiles
                    # (even head cols 0-63, odd head cols 64-127) run together.
                    yps = psum.tile([128, 512], F32, tag="ps", name="ps")
                    for kt in range(QT_N):
                        for hs in range(2):
                            h = 2 * hp + hs
                            hb = hs * 64
                            nc.tensor.matmul(
                                yps[hb : hb + 64, :],
                                lhsT=V[b * QT_N + kt][:, h * 64 : (h + 1) * 64],
                                rhs=PTp[hs][kt][:],
                                start=(kt == 0),
                                stop=(kt == QT_N - 1),
                            )
                    nc.vector.tensor_copy(xT[ht][:], yps[:])

                # Output projection for this batch element.
                for qt in range(QT_N):
                    row0 = b * S + qt * 128
                    ot = opool.tile([128, NLOC], F32, tag="ot", name="ot")
                    for nlc in range(NL_N):
                        ps = psum.tile([128, 512], F32, tag="ps", name="ps")
                        if use_bias:
                            nc.tensor.matmul(
                                ps[:],
                                lhsT=ones[0:1, 0:128],
                                rhs=biases["bo"][0:1, nlc * 512 : (nlc + 1) * 512],
                                start=True,
                                stop=False,
                            )
                        for ft in range(FT_N):
                            nc.tensor.matmul(
                                ps[:],
                                lhsT=xT[ft][:, qt * 128 : (qt + 1) * 128],
                                rhs=wo[:, ft * NLOC + nlc * 512 : ft * NLOC + (nlc + 1) * 512],
                                start=(ft == 0 and not use_bias),
                                stop=(ft == FT_N - 1),
                            )
                        nc.vector.tensor_copy(
                            ot[:, nlc * 512 : (nlc + 1) * 512], ps[:]
                        )
                        if nlc % 2 == 1:
                            nc.sync.dma_start(
                                d_out[row0 : row0 + 128, (nlc - 1) * 512 : (nlc + 1) * 512],
                                ot[:, (nlc - 1) * 512 : (nlc + 1) * 512],
                            )

    nc.compile()
    return nc


def _prep_inputs(q, k, v, str_mat, attn_mask, Wq, bq, Wk, bk, Wv, bv, Wo, bo):
    bf = ml_dtypes.bfloat16
    # 1/DH folded into Wq (and bq): DH = 64 = 2^6, exact in floating point.
    wqT = np.ascontiguousarray((Wq / np.float32(DH)).T).astype(bf)
    wkT = np.ascontiguousarray(Wk.T).astype(bf)
    wvT = np.ascontiguousarray(Wv.T).astype(bf)
    woT = np.ascontiguousarray(Wo.T).astype(bf)

    # Pre-tile weights: [n*128, width] -> [128, n*width].
    def pretile(w):
        n = w.shape[0] // 128
        return np.ascontiguousarray(
            w.reshape(n, 128, w.shape[1]).transpose(1, 0, 2).reshape(128, -1)
        )

    wqt = pretile(wqT)
    wkt = pretile(wkT)
    wvt = pretile(wvT)
    wot = pretile(woT)

    bqr = (bq[None, :] / np.float32(DH)).astype(bf)
    bkr = bk[None, :].astype(bf)
    bvr = bv[None, :].astype(bf)
    bor = bo[None, :].astype(bf)
    ident = np.eye(128, dtype=bf)
    onesr = np.ones((1, 512), dtype=bf)

    q16 = np.asarray(q).astype(bf)
    k16 = np.asarray(k).astype(bf)
    v16 = np.asarray(v).astype(bf)

    def pretile_T(x):
        # [R, NLOC] -> [128, KT_N*R] with [p, i*R+r] = x[r, i*128+p]
        return np.ascontiguousarray(
            x.reshape(R, KT_N, 128).transpose(2, 1, 0).reshape(128, KT_N * R)
        )
    strf = np.asarray(str_mat, dtype=np.float32)
    maskf = np.asarray(attn_mask).astype(np.float32)

    in_maps = []
    for c in range(NCORES):
        sl = slice(c * BL, (c + 1) * BL)
        # [BL, S, S] -> [128, BL*QT_N*S] with [p, (b*4+qt)*S+col].
        strt = np.ascontiguousarray(
            strf[sl].reshape(BL * QT_N, 128, S).transpose(1, 0, 2).reshape(128, -1)
        )
        maskt = np.ascontiguousarray(
            maskf[sl].reshape(BL * QT_N, 128, S).transpose(1, 0, 2).reshape(128, -1)
        )
        in_maps.append(
            {
                "q": pretile_T(q16[sl].reshape(R, NLOC)),
                "k": pretile_T(k16[sl].reshape(R, NLOC)),
                "v": pretile_T(v16[sl].reshape(R, NLOC)),
                "strm": strt,
                "maskf": maskt,
                "wqT": wqt,
                "wkT": wkt,
                "wvT": wvt,
                "woT": wot,
                "bqr": bqr,
                "bkr": bkr,
                "bvr": bvr,
                "bor": bor,
                "ident": ident,
                "onesr": onesr,
            }
        )
    return in_maps


def kernel(q, k, v, str_mat, attn_mask, Wq, bq, Wk, bk, Wv, bv, Wo, bo):
    use_bias = bool(
        np.any(np.asarray(bq))
        or np.any(np.asarray(bk))
        or np.any(np.asarray(bv))
        or np.any(np.asarray(bo))
    )
    key = ("nc", use_bias)
    if key not in _CACHE:
        _CACHE[key] = _build(use_bias)
    nc = _CACHE[key]
    in_maps = _prep_inputs(
        q, k, v, str_mat, attn_mask, Wq, bq, Wk, bk, Wv, bv, Wo, bo
    )
    res = run_bass_kernel_spmd(nc, in_maps, core_ids=list(range(NCORES)))
    out = np.empty((B, S, NLOC), dtype=np.float32)
    for c in range(NCORES):
        out[c * BL : (c + 1) * BL] = res.results[c]["out"].reshape(BL, S, NLOC)
    return out
